# revision 21
# baseline (speedup 1.0000x reference)
"""Trainium2 Bass kernel for nn_AttnEmo: cross-attention + residual + LayerNorm.

Sharding: pure data-parallel over batch B=8 across the 8 NeuronCores
(core b processes batch element b; no collectives needed).

Per-core math (S=T=2048, E=512), with host-side weight fusion:
  W_qk  = Wq.T @ Wk        -> logits = (x @ W_qk) @ emo.T  (k-proj eliminated)
  W_voT = (Wo @ Wv).T      -> attn   = softmax(logits) @ (emo @ W_voT)
                              (output projection folded into v-projection)
  bias terms fold exactly: bk/bq row-constant logit terms are softmax-
  invariant (dropped); the bq column term r[t] = (bq@Wk)@emo.T folds into
  the host mask tensor; bv contributes ub = bv@Wo.T added to a2 (softmax
  weights sum to 1).

On-chip layout (contract over the PE partition dim throughout):
  qqT [e,S] bf16 via lhsT=W_qk, rhs=xT;  u [T,e] fp8e4 via lhsT=emoT,
  rhs=W_voT;  scores psum in [s,1024] halves (3-deep pool); DVE adds mq
  (-1e18 at masked positions, bq column bias at kept ones) into an SBUF
  f32 tile, one full-row DVE reduce_max(negate) -> bias, one ACT Exp with
  accum_out row-sums -> w bf16; xbar-DMA transpose -> wT [t,16,128]; ACT
  cast -> fp8e4 (no act table, so no Exp-table thrash); attention matmul
  runs fp8 DoubleRow (two t-tiles per instruction, 2x PE throughput):
     lhsT = wT8[:, 2c:2c+2, :], rhs = u8[:, 2c:2c+2, :]
  epilogue: ACT copy(attn*rs) -> gpsimd +x -> DVE bn_stats/bn_aggr, sqrt
  batched one ACT call per 4 blocks (avoids Exp<->Sqrt act-table thrash),
  DVE normalize, gpsimd final residual.

Software pipeline is 7 cycles deep; emission order per cycle keeps every
engine queue head dependent only on prior-cycle producers (in-order
engine queues otherwise serialize on same-cycle cross-engine waits).
Known remaining headroom (measured 207.6us vs 232us baseline): head+proj
phase ~57us (input DMA not fully overlapped with projections), s-loop
~8.7us/block vs ~6.5 engine-busy bound, tail ~23us.
"""
import sys

sys.path.insert(0, "/opt/trn_rl_repo")
import numpy as np
import ml_dtypes

import concourse.bass as bass
from concourse import bacc
import concourse.mybir as mybir
import concourse.tile as tile
from concourse.bass_utils import run_bass_kernel_spmd
from contextlib import ExitStack

BF = ml_dtypes.bfloat16
S = 2048
T = 2048
E = 512
P = 128
SB = S // P   # 16 s-blocks
TB = T // P   # 16 t-blocks
EB = E // P   # 4 e-blocks
EPS = 1e-6
MASK_BIG = np.float32(-1e18)


def build_graph(has_bv, has_gb, use_fp8=True):
    f32, bf16 = mybir.dt.float32, mybir.dt.bfloat16
    fp8 = mybir.dt.float8e4
    nc = bacc.Bacc()

    x_ext = nc.declare_dram_parameter("x", [S, E], f32, isOutput=False)
    xT_ext = nc.declare_dram_parameter("xT", [E, S], bf16, isOutput=False)
    emoT_ext = nc.declare_dram_parameter("emoT", [E, T], bf16, isOutput=False)
    mq_ext = nc.declare_dram_parameter("mq", [S, T], bf16, isOutput=False)
    wqk_ext = nc.declare_dram_parameter("wqk", [E, E], bf16, isOutput=False)
    wvoT_ext = nc.declare_dram_parameter("wvoT", [E, E], bf16, isOutput=False)
    ub_ext = nc.declare_dram_parameter("ub", [E], f32, isOutput=False) if has_bv else None
    gamma_ext = nc.declare_dram_parameter("gamma", [E], f32, isOutput=False) if has_gb else None
    beta_ext = nc.declare_dram_parameter("beta", [E], f32, isOutput=False) if has_gb else None
    out_ext = nc.declare_dram_parameter("out", [S, E], f32, isOutput=True)

    OP = mybir.AluOpType
    AF = mybir.ActivationFunctionType
    PM = mybir.MatmulPerfMode.DoubleRow if use_fp8 else None

    with tile.TileContext(nc) as tc, ExitStack() as ctx:
        consts = ctx.enter_context(tc.tile_pool(name="consts", bufs=1))
        persist = ctx.enter_context(tc.tile_pool(name="persist", bufs=1))
        psS = ctx.enter_context(tc.tile_pool(name="psS", bufs=2, space="PSUM"))
        psA = ctx.enter_context(tc.tile_pool(name="psA", bufs=4, space="PSUM"))
        mpool = ctx.enter_context(tc.tile_pool(name="mpool", bufs=3))
        xpool = ctx.enter_context(tc.tile_pool(name="xpool", bufs=7))
        sblk = ctx.enter_context(tc.tile_pool(name="sblk", bufs=2))
        wtp = ctx.enter_context(tc.tile_pool(name="wtp", bufs=2))
        epi = ctx.enter_context(tc.tile_pool(name="epi", bufs=6))
        stat = ctx.enter_context(tc.tile_pool(name="stat", bufs=8))

        # ---- head: load in consumption order
        wqk_sb = consts.tile([P, EB, E], bf16)
        nc.sync.dma_start(out=wqk_sb, in_=wqk_ext.rearrange("(ki p) j -> p ki j", p=P))
        xT_sb = persist.tile([P, EB, S], bf16)
        xT_src = xT_ext.rearrange("(ki p) s -> p ki s", p=P)
        emoT_sb = persist.tile([P, EB, T], bf16)
        emoT_src = emoT_ext.rearrange("(ki p) s -> p ki s", p=P)
        for ki in range(EB):
            eng = nc.sync if ki % 2 == 0 else nc.scalar
            eng.dma_start(out=xT_sb[:, ki, :], in_=xT_src[:, ki, :])
        wvoT_sb = consts.tile([P, EB, E], bf16)
        nc.scalar.dma_start(out=wvoT_sb, in_=wvoT_ext.rearrange("(ki p) j -> p ki j", p=P))
        for ki in range(EB):
            eng = nc.scalar if ki % 2 == 0 else nc.sync
            eng.dma_start(out=emoT_sb[:, ki, :], in_=emoT_src[:, ki, :])

        def bcast_load(ext):  # [E] f32 -> [P, E] broadcast across partitions
            ap = ext.ap() if hasattr(ext, "ap") and callable(ext.ap) else ext
            t = consts.tile([P, E], f32, name=f"bc_{ap.tensor.name}")
            src = bass.AP(tensor=ap.tensor, offset=ap.offset,
                          ap=[[0, P]] + list(ap.ap))
            nc.gpsimd.dma_start(out=t, in_=src)
            return t

        eps_sb = consts.tile([P, 1], f32)
        nc.vector.memset(eps_sb, EPS)
        neghalf_sb = consts.tile([P, 1], f32)
        nc.vector.memset(neghalf_sb, -0.5)
        one_sb = consts.tile([P, 1], f32)
        nc.vector.memset(one_sb, 1.0)
        ub_bc = bcast_load(ub_ext) if has_bv else None
        gamma_bc = bcast_load(gamma_ext) if has_gb else None
        beta_bc = bcast_load(beta_ext) if has_gb else None

        # mask + x prefetch tiles (2 s-blocks per DMA)
        mq_tiles = {}
        x_tiles = {}

        def prefetch_pair(k):  # load s-blocks k, k+1
            mt = mpool.tile([P, 2, T], bf16, name=f"mq{k}", tag="mq")
            nc.gpsimd.dma_start(
                out=mt,
                in_=mq_ext[k * P:(k + 2) * P, :].rearrange("(b p) t -> p b t", p=P))
            mq_tiles[k] = mt
            mq_tiles[k + 1] = mt
            xt = xpool.tile([P, 2, E], f32, name=f"x{k}", tag="x")
            nc.sync.dma_start(
                out=xt,
                in_=x_ext[k * P:(k + 2) * P, :].rearrange("(b p) e -> p b e", p=P))
            x_tiles[k] = xt
            x_tiles[k + 1] = xt

        prefetch_pair(0)
        prefetch_pair(2)

        # ---- projections
        qqT_sb = persist.tile([P, EB, S], bf16)
        for eb in range(EB):
            for sc in range(S // 512):
                ps = psA.tile([P, 512], f32, tag="ps512", name=f"qq{eb}_{sc}")
                for ki in range(EB):
                    nc.tensor.matmul(
                        ps, lhsT=wqk_sb[:, ki, eb * P:(eb + 1) * P],
                        rhs=xT_sb[:, ki, sc * 512:(sc + 1) * 512],
                        start=(ki == 0), stop=(ki == EB - 1))
                dst = qqT_sb[:, eb, sc * 512:(sc + 1) * 512]
                nc.vector.tensor_copy(out=dst, in_=ps)

        u_sb = persist.tile([P, TB, E], fp8 if use_fp8 else bf16)
        for tb in range(TB):
            ps = psA.tile([P, 512], f32, tag="ps512", name=f"u{tb}")
            for ki in range(EB):
                nc.tensor.matmul(
                    ps, lhsT=emoT_sb[:, ki, tb * P:(tb + 1) * P],
                    rhs=wvoT_sb[:, ki, :],
                    start=(ki == 0), stop=(ki == EB - 1))
            if has_bv:
                nc.gpsimd.scalar_tensor_tensor(
                    out=u_sb[:, tb, :], in0=ps, scalar=1.0, in1=ub_bc,
                    op0=OP.mult, op1=OP.add)
            else:
                nc.scalar.copy(out=u_sb[:, tb, :], in_=ps)

        # ---- attention s-loop, software-pipelined 4 cycles deep.
        # Emission per cycle k is ordered so that no engine's queue head
        # waits on a same-cycle cross-engine producer:
        #   stage 0 (cycle k):   scores(k), mask+max (DVE), exp (ACT),
        #                        transpose (sync)
        #   stage 1 (cycle k+1): cast DMA (gpsimd), row-sum+recip (DVE),
        #                        attn matmul (PE)
        #   stage 2 (cycle k+2): a2 = attn*rs (ACT) + x (gpsimd),
        #                        bn_stats/aggr (DVE), sqrt (ACT, late)
        #   stage 3 (cycle k+3): stde/rstd/normalize (DVE), +x (gpsimd),
        #                        out DMA (ACT)
        wT8 = {}
        wT_bf = {}
        exp_sums = {}
        rs_all = {}
        attn_ps = {}
        a2_all = {}
        mv_all = {}
        std_all = {}

        def softmax_block(k):
            mt = mq_tiles.pop(k)
            mrow = mt[:, k % 2, :]
            negm = sblk.tile([P, T], f32, name=f"negm{k}", tag="negm")
            for h in range(2):
                ps = psS.tile([P, 1024], f32, tag="sc", name=f"sc{k}_{h}")
                for sub in range(2):
                    for ki in range(EB):
                        nc.tensor.matmul(
                            ps[:, sub * 512:(sub + 1) * 512],
                            lhsT=qqT_sb[:, ki, k * P:(k + 1) * P],
                            rhs=emoT_sb[:, ki, h * 1024 + sub * 512:
                                        h * 1024 + (sub + 1) * 512],
                            start=(ki == 0), stop=(ki == EB - 1))
                mh = negm[:, h * 1024:(h + 1) * 1024]
                nc.vector.tensor_tensor(out=mh, in0=ps,
                                        in1=mrow[:, h * 1024:(h + 1) * 1024],
                                        op=OP.add)
            mn = stat.tile([P, 1], f32, name=f"mn{k}", tag="mn")
            nc.vector.reduce_max(mn, negm, axis=mybir.AxisListType.X,
                                 negate=True)
            w_bf = sblk.tile([P, T], bf16, name=f"wbf{k}", tag="wbf")
            sums = stat.tile([P, 1], f32, name=f"sums{k}", tag="sums")
            nc.scalar.activation(out=w_bf, in_=negm, func=AF.Exp,
                                 bias=mn, scale=1.0, accum_out=sums)
            exp_sums[k] = sums
            wT = wtp.tile([P, TB, P], bf16, name=f"wT{k}", tag="wT")
            nc.sync.dma_start_transpose(out=wT, in_=w_bf)
            wT_bf[k] = wT

        def finish_softmax(k):  # DVE: reciprocal of the row sums
            rs = stat.tile([P, 1], f32, name=f"rs{k}", tag="rs")
            nc.vector.reciprocal(rs, exp_sums.pop(k))
            rs_all[k] = rs

        def cast_block(k):  # wT bf16 -> fp8 (ACT copy)
            wT = wT_bf.pop(k)
            if use_fp8:
                w8 = wtp.tile([P, TB, P], fp8, name=f"w8_{k}", tag="w8")
                nc.scalar.copy(out=w8, in_=wT)
                wT8[k] = w8
            else:
                wT8[k] = wT

        def attn_block(k):
            aps = psA.tile([P, E], f32, tag="ps512", name=f"aps{k}")
            w8 = wT8.pop(k)
            if use_fp8:
                for c in range(TB // 2):
                    nc.tensor.matmul(
                        aps, lhsT=w8[:, 2 * c:2 * c + 2, :],
                        rhs=u_sb[:, 2 * c:2 * c + 2, :],
                        start=(c == 0), stop=(c == TB // 2 - 1),
                        perf_mode=PM)
            else:
                for tb in range(TB):
                    nc.tensor.matmul(
                        aps, lhsT=w8[:, tb, :], rhs=u_sb[:, tb, :],
                        start=(tb == 0), stop=(tb == TB - 1))
            attn_ps[k] = aps

        def a2_block(k):  # ACT: attn*rs from psum; gpsimd: + x residual
            aps = attn_ps.pop(k)
            a2 = epi.tile([P, E], f32, name=f"a2{k}", tag="a2")
            nc.scalar.activation(out=a2, in_=aps, func=AF.Copy,
                                 scale=rs_all.pop(k))
            nc.gpsimd.tensor_add(out=a2, in0=a2, in1=x_tiles[k][:, k % 2, :])
            if has_bv:
                nc.gpsimd.tensor_add(out=a2, in0=a2, in1=ub_bc)
            a2_all[k] = a2

        mvg_all = {}
        rstdg_all = {}

        def bn_block(k):  # DVE: stats into the group-of-4 tile
            a2 = a2_all[k]
            g = k // 4
            if k % 4 == 0:
                mvg_all[g] = stat.tile([P, 4, 2], f32, name=f"mvg{g}",
                                       tag="mvg")
            st6 = stat.tile([P, 6], f32, name=f"st6{k}", tag="st6")
            nc.vector.bn_stats(out=st6, in_=a2)
            nc.vector.bn_aggr(out=mvg_all[g][:, k % 4, :], in_=st6)

        def sqrt_group(g):  # one ACT sqrt + DVE eps/recip per 4 blocks
            mvg = mvg_all[g]
            stdg = stat.tile([P, 4], f32, name=f"stdg{g}", tag="stdg")
            nc.scalar.sqrt(stdg, mvg[:, :, 1])
            nc.vector.tensor_scalar(out=stdg, in0=stdg, scalar1=eps_sb,
                                    scalar2=None, op0=OP.add)
            rstdg = stat.tile([P, 4], f32, name=f"rstdg{g}", tag="rstdg")
            nc.vector.reciprocal(rstdg, stdg)
            rstdg_all[g] = rstdg

        z_all = {}

        def epi_block(k):  # DVE: normalize; gpsimd: final residual
            a2 = a2_all.pop(k)
            g = k // 4
            mvg = mvg_all[g]
            rstdg = rstdg_all[g]
            xt = x_tiles.pop(k)
            x_blk = xt[:, k % 2, :]
            z = epi.tile([P, E], f32, name=f"z{k}", tag="zz")
            nc.vector.tensor_scalar(out=z, in0=a2,
                                    scalar1=mvg[:, k % 4, 0:1],
                                    scalar2=rstdg[:, k % 4:k % 4 + 1],
                                    op0=OP.subtract, op1=OP.mult)
            if has_gb:
                nc.vector.tensor_mul(out=z, in0=z, in1=gamma_bc)
                nc.vector.tensor_add(out=z, in0=z, in1=beta_bc)
            nc.gpsimd.tensor_add(out=z, in0=z, in1=x_blk)
            z_all[k] = z

        def out_block(k):  # ACT-issued DMA, one cycle after z is ready
            nc.scalar.dma_start(out=out_ext[k * P:(k + 1) * P, :],
                                in_=z_all.pop(k))

        for k in range(SB):
            if k >= 1:
                cast_block(k - 1)
            if k >= 2:
                a2_block(k - 2)
            if k >= 1:
                finish_softmax(k - 1)
            if k >= 3:
                bn_block(k - 3)
                if (k - 3) % 4 == 3:
                    sqrt_group((k - 3) // 4)
            if k >= 6:
                epi_block(k - 6)
            if k >= 7:
                out_block(k - 7)
            softmax_block(k)
            if k >= 1:
                attn_block(k - 1)
            if k % 2 == 0 and k + 4 < SB:
                prefetch_pair(k + 4)
        cast_block(SB - 1)
        finish_softmax(SB - 1)
        attn_block(SB - 1)
        a2_block(SB - 2)
        a2_block(SB - 1)
        bn_block(SB - 3)
        bn_block(SB - 2)
        bn_block(SB - 1)
        sqrt_group(SB // 4 - 1)
        for j in range(SB - 6, SB):
            epi_block(j)
        for j in range(SB - 7, SB):
            out_block(j)

    nc.finalize()
    return nc


_GRAPH_CACHE = {}


def _get_graph(flags):
    if flags not in _GRAPH_CACHE:
        _GRAPH_CACHE[flags] = build_graph(*flags)
    return _GRAPH_CACHE[flags]


def make_in_maps(encoder_outputs, emotion, mask, Wq, bq, Wk, bk, Wv, bv, Wo,
                 gamma, beta):
    enc = np.asarray(encoder_outputs, np.float32)
    emo = np.asarray(emotion, np.float32)
    mask = np.asarray(mask)
    B = enc.shape[0]
    Wq = np.asarray(Wq, np.float32)
    Wk = np.asarray(Wk, np.float32)
    Wv = np.asarray(Wv, np.float32)
    Wo = np.asarray(Wo, np.float32)
    bq = np.asarray(bq, np.float32)
    bv = np.asarray(bv, np.float32)
    gamma = np.asarray(gamma, np.float32)
    beta = np.asarray(beta, np.float32)

    has_bq = bool(np.any(bq))
    has_bv = bool(np.any(bv))
    has_gb = not (np.allclose(gamma, 1.0) and np.allclose(beta, 0.0))

    wqk = np.ascontiguousarray(Wq.T @ Wk).astype(BF)
    wvoT = np.ascontiguousarray((Wo @ Wv).T).astype(BF)
    ub = (bv @ Wo.T).astype(np.float32) if has_bv else None

    in_maps = []
    for b in range(B):
        if has_bq:
            r = ((bq @ Wk) @ emo[b].T).astype(np.float32)  # [T]
            mq = np.where(mask[b], MASK_BIG, r[None, :])
        else:
            mq = np.where(mask[b], MASK_BIG, np.float32(0.0))
        m = {
            "x": enc[b],
            "xT": np.ascontiguousarray(enc[b].T).astype(BF),
            "emoT": np.ascontiguousarray(emo[b].T).astype(BF),
            "mq": mq.astype(BF),
            "wqk": wqk, "wvoT": wvoT,
        }
        if has_bv:
            m["ub"] = ub
        if has_gb:
            m["gamma"] = gamma
            m["beta"] = beta
        in_maps.append(m)
    return (has_bv, has_gb), in_maps


def kernel(encoder_outputs, emotion, mask, Wq, bq, Wk, bk, Wv, bv, Wo,
           gamma, beta):
    flags, in_maps = make_in_maps(encoder_outputs, emotion, mask, Wq, bq, Wk,
                                  bk, Wv, bv, Wo, gamma, beta)
    nc = _get_graph(flags)
    B = len(in_maps)
    res = run_bass_kernel_spmd(nc, in_maps, list(range(B)))
    out = np.stack([np.asarray(res.results[i]["out"], np.float32)
                    for i in range(B)])
    return out
